# revision 17
# baseline (speedup 1.0000x reference)
"""Trainium2 Bass kernel for GCN(x2) + MHA + mean + FC, sharded over 8 NeuronCores.

Sharding: 1D row partition of the 4096 nodes (512 rows/core). Each core holds
the column slice adj_hat[:, r*512:(r+1)*512] of the symmetric A+I (by symmetry
equal to its row block transposed), all of x, and replicated weights.
Cross-core exchanges (on-device AllGather): degree vector, scaled GCN1 output,
K (bf16) and V (fp8) for all heads.

Numerics: adjacency is {0,1,2} so fp8e4 is exact; message-passing matmuls run
as fp8 DoubleRow (2 contraction rows/pass), scores in bf16, softmax probs and
V in fp8 (the softmax here is near-uniform, and numerator/denominator share
the same quantized probs, so fp8 quantization largely cancels). All PSUM
accumulation is fp32. Host does only slicing/dtype prep and an 8-way sum of
[2]-vector partials.
"""
import sys
sys.path.insert(0, "/opt/trn_rl_repo")
import numpy as np
import ml_dtypes

N = 4096
NC_ = 8
R = N // NC_          # 512 rows per core
KB = N // 128         # 32 node chunks
KP = KB // 2          # 16 chunk pairs (fp8 DoubleRow)
F_IN = 128
G1 = 128
G2 = 512
HEADS = 4
HD = G2 // HEADS      # 128
ET = G2 // 128        # 4 tiles of the 512-dim embedding

_cache = {}


def _build(sim1=False):
    from concourse import bass, bacc, tile, mybir

    f32 = mybir.dt.float32
    bf16 = mybir.dt.bfloat16
    f8 = mybir.dt.float8e4
    AF = mybir.ActivationFunctionType
    ALU = mybir.AluOpType
    AX = mybir.AxisListType
    PM = mybir.MatmulPerfMode

    nc = bacc.Bacc("TRN2", target_bir_lowering=False, debug=False,
                   num_devices=1 if sim1 else NC_)

    # ---- kernel I/O (per-core shards supplied via in_maps) ----
    adj_d = nc.dram_tensor("adjc", [N, R], f8, kind="ExternalInput")
    x_d = nc.dram_tensor("x", [N, F_IN], bf16, kind="ExternalInput")
    w1_d = nc.dram_tensor("w1", [F_IN, G1], bf16, kind="ExternalInput")
    b1_d = nc.dram_tensor("b1", [G1], f32, kind="ExternalInput")
    w2_d = nc.dram_tensor("w2", [G1, G2], bf16, kind="ExternalInput")
    b2_d = nc.dram_tensor("b2", [G2], f32, kind="ExternalInput")
    win_d = nc.dram_tensor("win", [G2, 3 * G2], bf16, kind="ExternalInput")
    bin_d = nc.dram_tensor("bin", [3 * G2], f32, kind="ExternalInput")
    wo_d = nc.dram_tensor("wo", [G2, G2], f32, kind="ExternalInput")
    bo_d = nc.dram_tensor("bo", [G2], f32, kind="ExternalInput")
    fcw_d = nc.dram_tensor("fcw", [G2, 2], f32, kind="ExternalInput")
    fcb_d = nc.dram_tensor("fcb", [2], f32, kind="ExternalInput")
    out_d = nc.dram_tensor("outp", [1, 2], f32, kind="ExternalOutput")

    RG = [list(range(NC_))]
    inv_sqrt_hd = 1.0 / float(np.sqrt(HD))

    with tile.TileContext(nc) as tc:
        with tc.tile_pool(name="wts", bufs=1) as wts, \
             tc.tile_pool(name="adj", bufs=1) as adjp, \
             tc.tile_pool(name="xstg", bufs=1) as xstgp, \
             tc.tile_pool(name="xs", bufs=1) as xsp, \
             tc.tile_pool(name="x1s", bufs=1) as x1sp, \
             tc.tile_pool(name="act", bufs=1) as actp, \
             tc.tile_pool(name="kvq", bufs=1) as kvp, \
             tc.tile_pool(name="kvg", bufs=1) as kvgp, \
             tc.tile_pool(name="pt", bufs=4) as ptp, \
             tc.tile_pool(name="small", bufs=2) as smp, \
             tc.tile_pool(name="psBig", bufs=2, space="PSUM") as psB, \
             tc.tile_pool(name="psMid", bufs=2, space="PSUM") as psM, \
             tc.tile_pool(name="psSm", bufs=2, space="PSUM") as psS, \
             tc.tile_pool(name="dram", bufs=1, space="DRAM") as drp:

            # ================= prologue: DMAs =================
            # adjacency: 4 big strided DMAs on sync, 8 node-chunks each
            adjt = adjp.tile([128, KB, R], f8)
            for i in range(4):
                nc.sync.dma_start(
                    adjt[:, i * 8:(i + 1) * 8, :],
                    adj_d[i * 1024:(i + 1) * 1024, :]
                    .rearrange("(c p) r -> p c r", p=128))
            # x: 2 strided DMAs on scalar
            xstage = xstgp.tile([128, KB, F_IN], bf16)
            for i in range(2):
                nc.sync.dma_start(
                    xstage[:, i * 16:(i + 1) * 16, :],
                    x_d[i * 2048:(i + 1) * 2048, :]
                    .rearrange("(c p) f -> p c f", p=128))
            # weights (host pre-converted to bf16 where used as bf16)
            w1s = wts.tile([128, G1], bf16)
            nc.scalar.dma_start(w1s[:], w1_d[:, :])
            w2s = wts.tile([128, G2], bf16)
            nc.scalar.dma_start(w2s[:], w2_d[:, :])
            # biases
            b1row = wts.tile([1, G1], f32)
            nc.sync.dma_start(b1row[:], b1_d[:])
            b1bc = wts.tile([128, G1], f32)
            nc.gpsimd.partition_broadcast(b1bc[:], b1row[:])
            b2col = wts.tile([128, ET], f32)
            nc.sync.dma_start(b2col[:], b2_d[:].rearrange("(c p) -> p c", p=128))
            bincol = wts.tile([128, 12], f32)
            nc.sync.dma_start(bincol[:], bin_d[:].rearrange("(c p) -> p c", p=128))
            bocol = wts.tile([128, ET], f32)
            nc.sync.dma_start(bocol[:], bo_d[:].rearrange("(c p) -> p c", p=128))
            bo8s = wts.tile([128, ET], f32)
            nc.vector.tensor_scalar_mul(bo8s[:], bocol[:], 1.0 / NC_)
            # fp8 all-ones stationary for DoubleRow column sums (M=128 keeps
            # walrus happy; every output partition gets the same sums)
            ones8 = wts.tile([128, 2, 128], f8)
            nc.vector.memset(ones8[:], 1.0)

            # ================= degree (PE, fp8 DoubleRow) =================
            ps_deg = psS.tile([128, G2], f32, tag="sm")
            for t in range(KP):
                nc.tensor.matmul(ps_deg[:], ones8[:], adjt[:, 2 * t:2 * t + 2, :],
                                 start=(t == 0), stop=(t == KP - 1),
                                 perf_mode=PM.DoubleRow, skip_group_check=True)
            sq = smp.tile([1, G2], f32, tag="sq")
            nc.scalar.activation(sq[:], ps_deg[0:1, :], AF.Sqrt)
            dloc = wts.tile([1, G2], f32)
            nc.vector.reciprocal(dloc[:], sq[:])
            # broadcast own-row scale over free dim (for output-side scaling)
            dbc = wts.tile([128, G2], f32)
            nc.gpsimd.partition_broadcast(dbc[:], dloc[:])

            # AG1: gather d across cores
            dgin = drp.tile([1, G2], f32, tag="dgin")
            dg_out = nc.dram_tensor("dg_out", [NC_, G2], f32, kind="Internal",
                                    addr_space="Shared")
            nc.sync.dma_start(dgin[:], dloc[:])
            if sim1:
                nc.sync.dma_start(dg_out[:, :],
                                  dgin[:].to_broadcast([NC_, G2]))
            else:
                nc.gpsimd.collective_compute(
                    "AllGather", ALU.bypass, replica_groups=RG,
                    ins=[dgin.opt()], outs=[dg_out.ap()])
            # all-node column scales [128, 32] and own-rows per-partition [128, 4]
            dcol = wts.tile([128, KB], f32)
            nc.sync.dma_start(
                dcol[:], dg_out[:, :].rearrange("r (c p) -> p r c", p=128))
            down = wts.tile([128, ET], f32)
            nc.scalar.dma_start(
                down[:], dgin[:].rearrange("a (c p) -> p a c", p=128))

            # ================= GCN1 =================
            xst = xsp.tile([128, KB, F_IN], f8)
            for kb in range(KB):
                if kb % 2 == 0:
                    nc.vector.tensor_scalar_mul(xst[:, kb, :], xstage[:, kb, :],
                                                dcol[:, kb:kb + 1])
                else:
                    nc.scalar.activation(xst[:, kb, :], xstage[:, kb, :],
                                         AF.Identity, scale=dcol[:, kb:kb + 1])
            ps_s1 = psM.tile([128, R], f32, tag="mid")
            for t in range(KP):
                nc.tensor.matmul(ps_s1[:], xst[:, 2 * t:2 * t + 2, :],
                                 adjt[:, 2 * t:2 * t + 2, :],
                                 start=(t == 0), stop=(t == KP - 1),
                                 perf_mode=PM.DoubleRow, skip_group_check=True)
            s1t = actp.tile([128, R], bf16, tag="s1t")
            nc.vector.tensor_mul(s1t[:], ps_s1[:], dbc[:])
            # x1 = relu(s1.T @ W1 + b1), node-major blocks [128, 4, 128]
            x1ps = psM.tile([128, R], f32, tag="mid")
            for c in range(ET):
                nc.tensor.matmul(x1ps[:, c * 128:(c + 1) * 128],
                                 s1t[:, c * 128:(c + 1) * 128], w1s[:],
                                 start=True, stop=True, skip_group_check=True)
            x1b = smp.tile([128, R], f32, tag="x1b")
            for c in range(ET):
                nc.vector.tensor_add(x1b[:, c * 128:(c + 1) * 128],
                                     x1ps[:, c * 128:(c + 1) * 128], b1bc[:])
            # fold the own-node GCN2 column scale in before the gather:
            # d>0 so d*relu(x) == relu(d*x)
            x1s8 = smp.tile([128, ET, G1], f8, tag="x1s8")
            for c in range(ET):
                nc.scalar.activation(x1s8[:, c, :], x1b[:, c * 128:(c + 1) * 128],
                                     AF.Relu, scale=down[:, c:c + 1])

            # AG2: gather scaled x1 (fp8, node-major)
            x1i = drp.tile([R, G1], f8, tag="x1i")
            x1g = nc.dram_tensor("x1g", [N, G1], f8, kind="Internal",
                                 addr_space="Shared")
            nc.scalar.dma_start(
                x1i[:].rearrange("(c p) g -> p c g", p=128), x1s8[:])
            if sim1:
                nc.scalar.dma_start(
                    x1g[:, :].rearrange("(r q) g -> r q g", r=NC_),
                    x1i[:].unsqueeze(0).to_broadcast([NC_, R, G1]))
            else:
                nc.gpsimd.collective_compute(
                    "AllGather", ALU.bypass, replica_groups=RG,
                    ins=[x1i.opt()], outs=[x1g.ap()])
            x1st = x1sp.tile([128, KB, G1], f8)
            nc.sync.dma_start(
                x1st[:], x1g[:, :].rearrange("(c p) g -> p c g", p=128))

            # in_proj weights: needed from the QKV phase on; loading here
            # overlaps the AG2 round-trip
            winb = []
            for c in range(ET):
                wb = wts.tile([128, 3 * G2], bf16, tag=f"winb{c}")
                nc.sync.dma_start(wb[:], win_d[c * 128:(c + 1) * 128, :])
                winb.append(wb)

            # ================= GCN2 =================
            ps_s2 = psM.tile([128, R], f32, tag="mid")
            for t in range(KP):
                nc.tensor.matmul(ps_s2[:], x1st[:, 2 * t:2 * t + 2, :],
                                 adjt[:, 2 * t:2 * t + 2, :],
                                 start=(t == 0), stop=(t == KP - 1),
                                 perf_mode=PM.DoubleRow, skip_group_check=True)
            s2t = actp.tile([128, R], bf16, tag="s2t")
            nc.vector.tensor_mul(s2t[:], ps_s2[:], dbc[:])
            # x2T tiles [g2-chunk 128, node 512]
            x2t = []
            for c in range(ET):
                psx = psM.tile([128, R], f32, tag="mid")
                nc.tensor.matmul(psx[:], w2s[:, c * 128:(c + 1) * 128], s2t[:],
                                 start=True, stop=True, skip_group_check=True)
                xt = actp.tile([128, R], bf16, tag=f"x2_{c}")
                nc.scalar.activation(xt[:], psx[:], AF.Identity,
                                     bias=b2col[:, c:c + 1])
                x2t.append(xt)

            # ============ K (all heads) -> AG3K -> loads; V -> AG3V; Q ============
            ktall = kvp.tile([128, HEADS, R], f8, tag="ktall")
            for h in range(HEADS):
                psk = psM.tile([128, R], f32, tag="mid")
                for c in range(ET):
                    nc.tensor.matmul(psk[:],
                                     winb[c][:, G2 + h * 128:G2 + (h + 1) * 128],
                                     x2t[c][:], start=(c == 0), stop=(c == ET - 1),
                                     skip_group_check=True)
                nc.scalar.activation(ktall[:, h, :], psk[:], AF.Identity,
                                     bias=bincol[:, 4 + h:5 + h])
            kti = drp.tile([HEADS * 128, R], f8, tag="kti")
            ktg_d = nc.dram_tensor("ktg", [NC_ * HEADS * 128, R], f8,
                                   kind="Internal", addr_space="Shared")
            nc.sync.dma_start(
                kti[:].rearrange("(h p) n -> p h n", p=128), ktall[:])
            if sim1:
                nc.sync.dma_start(
                    ktg_d[:, :].rearrange("(r q) n -> r q n", r=NC_),
                    kti[:].unsqueeze(0).to_broadcast([NC_, HEADS * 128, R]))
            else:
                nc.gpsimd.collective_compute(
                    "AllGather", ALU.bypass, replica_groups=RG,
                    ins=[kti.opt()], outs=[ktg_d.ap()])
            ktg_v = ktg_d[:, :].rearrange("(r h p) n -> p h r n",
                                          h=HEADS, p=128)
            ktg = []
            for h in range(HEADS):
                kt = kvgp.tile([128, NC_, R], f8, tag=f"ktg{h}")
                nc.sync.dma_start(kt[:], ktg_v[:, h:h + 1, :, :])
                ktg.append(kt)

            vall = kvp.tile([128, HEADS, ET, HD], f8, tag="vall")
            for h in range(HEADS):
                psv = psM.tile([128, R], f32, tag="mid")
                for m in range(ET):
                    for c in range(ET):
                        nc.tensor.matmul(
                            psv[:, m * 128:(m + 1) * 128],
                            x2t[c][:, m * 128:(m + 1) * 128],
                            winb[c][:, 2 * G2 + h * 128:2 * G2 + (h + 1) * 128],
                            start=(c == 0), stop=(c == ET - 1),
                            skip_group_check=True)
                nc.vector.tensor_copy(
                    vall[:, h, :, :].rearrange("p c d -> p (c d)"), psv[:])
            vti = drp.tile([HEADS * R, HD], f8, tag="vti")
            vtg_d = nc.dram_tensor("vtg", [NC_ * HEADS * R, HD], f8,
                                   kind="Internal", addr_space="Shared")
            nc.scalar.dma_start(
                vti[:].rearrange("(h c p) d -> p h c d", p=128, h=HEADS), vall[:])
            if sim1:
                nc.scalar.dma_start(
                    vtg_d[:, :].rearrange("(r q) d -> r q d", r=NC_),
                    vti[:].unsqueeze(0).to_broadcast([NC_, HEADS * R, HD]))
            else:
                nc.gpsimd.collective_compute(
                    "AllGather", ALU.bypass, replica_groups=RG,
                    ins=[vti.opt()], outs=[vtg_d.ap()])
            # V for all heads: one DMA per rank (per-head reads need 4 AP dims)
            vtgall = kvgp.tile([128, NC_, HEADS * ET, HD], f8, tag="vtgall")
            for r in range(NC_):
                nc.gpsimd.dma_start(
                    vtgall[:, r:r + 1, :, :],
                    vtg_d[r * HEADS * R:(r + 1) * HEADS * R, :]
                    .rearrange("(q p) d -> p q d", p=128))

            def v_pair(h, t):
                # chunk pair (2t, 2t+1) of head h as a [128, 2, HD] lhsT view
                rr, c = (2 * t) // ET, (2 * t) % ET
                return vtgall[:, rr:rr + 1, h * ET + c:h * ET + c + 2, :] \
                    .rearrange("p a q d -> p (a q) d")

            # epilogue weights: load during attention
            wos = wts.tile([128, ET, G2], f32)
            nc.scalar.dma_start(
                wos[:], wo_d[:, :].rearrange("(c p) g -> p c g", p=128))
            fcws = wts.tile([128, ET, 2], f32)
            nc.scalar.dma_start(
                fcws[:], fcw_d[:, :].rearrange("(c p) t -> p c t", p=128))
            fcbrow = wts.tile([1, 2], f32)
            nc.scalar.dma_start(fcbrow[:], fcb_d[:])
            fcb8 = wts.tile([1, 2], f32)
            nc.vector.tensor_scalar_mul(fcb8[:], fcbrow[:], 1.0 / NC_)

            # Q per head (overlaps the gather traffic)
            qt = []
            for h in range(HEADS):
                psq = psM.tile([128, R], f32, tag="mid")
                for c in range(ET):
                    nc.tensor.matmul(psq[:], winb[c][:, h * 128:(h + 1) * 128],
                                     x2t[c][:], start=(c == 0), stop=(c == ET - 1),
                                     skip_group_check=True)
                q = kvp.tile([128, R], f8, tag=f"qt{h}")
                nc.scalar.activation(q[:], psq[:], AF.Identity,
                                     bias=bincol[:, h:h + 1])
                qt.append(q)

            # ================= attention =================
            # two heads interleaved so exp (Act) of one head overlaps the PE
            # matmuls of the other and the tensor engine stays ramped up
            z_sb = [None] * HEADS
            for h0 in (0, 2):
                pair_hs = (h0, h0 + 1)
                kslab = {h: ktg[h][:].rearrange("p r n -> p (r n)")
                         for h in pair_hs}
                ps_ctx, ps_den = {}, {}
                for h in pair_hs:
                    pc = psM.tile([128, R], f32, tag="mid", name=f"ctx{h}")
                    ps_ctx[h] = pc
                    pd = psS.tile([128, G2], f32, tag="sm", name=f"den{h}")
                    ps_den[h] = pd
                pend = {h: None for h in pair_hs}
                for t in range(KP):
                    for h in pair_hs:
                        ps_sc = psB.tile([128, 2, R], f32, tag="sc")
                        nc.tensor.matmul(
                            ps_sc[:, 0, :],
                            kslab[h][:, (2 * t) * 128:(2 * t + 1) * 128],
                            qt[h][:], start=True, stop=True,
                            skip_group_check=True)
                        nc.tensor.matmul(
                            ps_sc[:, 1, :],
                            kslab[h][:, (2 * t + 1) * 128:(2 * t + 2) * 128],
                            qt[h][:], start=True, stop=True,
                            skip_group_check=True)
                        pt = ptp.tile([128, 2, R], f8, tag="pt")
                        nc.scalar.activation(pt[:], ps_sc[:], AF.Exp,
                                             scale=inv_sqrt_hd)
                        if pend[h] is not None:
                            pp, tp = pend[h]
                            nc.tensor.matmul(ps_ctx[h][:], v_pair(h, tp),
                                             pp[:], start=(tp == 0), stop=False,
                                             perf_mode=PM.DoubleRow,
                                             skip_group_check=True)
                            nc.tensor.matmul(ps_den[h][:], ones8[:], pp[:],
                                             start=(tp == 0), stop=False,
                                             perf_mode=PM.DoubleRow,
                                             skip_group_check=True)
                        pend[h] = (pt, t)
                for h in pair_hs:
                    pp, tp = pend[h]
                    nc.tensor.matmul(ps_ctx[h][:], v_pair(h, tp),
                                     pp[:], start=False, stop=True,
                                     perf_mode=PM.DoubleRow,
                                     skip_group_check=True)
                    nc.tensor.matmul(ps_den[h][:], ones8[:], pp[:],
                                     start=False, stop=True,
                                     perf_mode=PM.DoubleRow,
                                     skip_group_check=True)
                for h in pair_hs:
                    # normalize rows and reduce over own queries
                    rden = smp.tile([1, R], f32, tag="rden")
                    nc.vector.reciprocal(rden[:], ps_den[h][0:1, :])
                    rbc = smp.tile([128, R], f32, tag="rbc")
                    nc.gpsimd.partition_broadcast(rbc[:], rden[:])
                    ctxn = smp.tile([128, R], f32, tag="ctxn")
                    nc.vector.tensor_mul(ctxn[:], ps_ctx[h][:], rbc[:])
                    zs = smp.tile([128, 1], f32, tag=f"z{h}")
                    nc.vector.tensor_reduce(zs[:], ctxn[:], axis=AX.X,
                                            op=ALU.add)
                    # V-bias fold: each of the R own queries contributes +bv
                    zb = smp.tile([128, 1], f32, tag=f"zb{h}")
                    nc.vector.tensor_scalar_mul(zb[:], bincol[:, 8 + h:9 + h],
                                                float(R))
                    zf = smp.tile([128, 1], f32, tag=f"zf{h}")
                    nc.vector.tensor_add(zf[:], zs[:], zb[:])
                    z_sb[h] = zf

            # ================= out_proj + mean + fc (partial) ==============
            u_sb = []
            for et in range(ET):
                psu = psM.tile([128, R], f32, tag="mid")
                for c in range(ET):
                    nc.tensor.matmul(psu[:, 0:1],
                                     wos[:, c, et * 128:(et + 1) * 128],
                                     z_sb[c][:], start=(c == 0), stop=(c == ET - 1),
                                     skip_group_check=True)
                ut = smp.tile([128, 1], f32, tag=f"u{et}")
                nc.scalar.activation(ut[:], psu[:, 0:1], AF.Identity,
                                     scale=1.0 / float(N),
                                     bias=bo8s[:, et:et + 1])
                u_sb.append(ut)
            ps_fc = psM.tile([128, R], f32, tag="mid")
            for c in range(ET):
                nc.tensor.matmul(ps_fc[0:1, 0:2], u_sb[c][:], fcws[:, c, :],
                                 start=(c == 0), stop=(c == ET - 1),
                                 skip_group_check=True)
            ores = smp.tile([1, 2], f32, tag="ores")
            nc.vector.tensor_add(ores[:], ps_fc[0:1, 0:2], fcb8[:])
            nc.sync.dma_start(out_d[:, :], ores[:])

    nc.compile()
    return nc


def kernel(**inputs):
    from concourse.bass_utils import run_bass_kernel_spmd

    if "nc" not in _cache:
        _cache["nc"] = _build()
    nc = _cache["nc"]

    adj = np.ascontiguousarray(inputs["adj_matrix"], dtype=np.float32)
    x = np.ascontiguousarray(inputs["node_features"]).astype(ml_dtypes.bfloat16)
    reps = {
        "x": x,
        "w1": np.ascontiguousarray(inputs["W1"]).astype(ml_dtypes.bfloat16),
        "b1": np.ascontiguousarray(inputs["b1"], np.float32),
        "w2": np.ascontiguousarray(inputs["W2"]).astype(ml_dtypes.bfloat16),
        "b2": np.ascontiguousarray(inputs["b2"], np.float32),
        "win": np.ascontiguousarray(inputs["in_proj_w"]).astype(ml_dtypes.bfloat16),
        "bin": np.ascontiguousarray(inputs["in_proj_b"], np.float32),
        "wo": np.ascontiguousarray(inputs["out_proj_w"], np.float32),
        "bo": np.ascontiguousarray(inputs["out_proj_b"], np.float32),
        "fcw": np.ascontiguousarray(inputs["fc_w"], np.float32),
        "fcb": np.ascontiguousarray(inputs["fc_b"], np.float32),
    }
    in_maps = []
    idx = np.arange(R)
    for r in range(NC_):
        cols = np.ascontiguousarray(adj[:, r * R:(r + 1) * R])
        cols[r * R + idx, idx] += 1.0   # A + I, this core's diagonal block
        # {0,1,2} adjacency is exact in fp8e4; quarters the dominant DMA
        in_maps.append({"adjc": cols.astype(ml_dtypes.float8_e4m3), **reps})

    res = run_bass_kernel_spmd(nc, in_maps, core_ids=list(range(NC_)))
    out = np.zeros(2, dtype=np.float64)
    for r in range(NC_):
        out += res.results[r]["outp"].reshape(2).astype(np.float64)
    return out.astype(np.float32)
